# revision 7
# baseline (speedup 1.0000x reference)
"""MoE-with-DeepGEMM kernel for 8 Trainium2 NeuronCores.

Problem: M=4096 tokens, D=2048 in-dim, H=2048 out-dim, E=8 experts.
    gate = softmax(x @ gate_w.T + gate_b)            # [M, E], fp32
    y    = (q8(x) @ q8(expert_w[e]).T) -> bf16       # [E, M, H]
    out  = sum_e gate[:, e, None] * y[e].astype(f32) # [M, H]

Strategy: data-parallel over tokens (M). Each of the 8 cores gets
M/8 = 512 tokens, all 8 experts' weights, and computes its output slice
independently — no collectives; the host concatenates the slices.

The PE issue rate is the wall: 1024 DoubleRow matmuls x ~213 ns plus 16
gating matmuls. The schedule keeps the PE issuing back-to-back and the
HBM supply matched to consumption order:
  - All inputs are HOST-PREARRANGED into flat per-partition layouts so
    every DMA descriptor row is wide (2KB+) and runs at full HBM rate
    (~350 GB/s); the baseline's rearranged transfers with 512B-1KB rows
    only reached ~200-250 GB/s in the critical first 20us.
  - e0 is restructured into m-split phases: phase A (mc0,1 x all hc)
    consumes w0 strictly k-major in 256KB chunks AS THEY ARRIVE;
    phase B (mc2,3) reuses w0 entirely from SBUF (zero DMA). The
    baseline's h-split phases interleaved consumption against arrival
    and stalled the PE mid-stream (HAM re-throttle cost ~4us).
  - 16 bf16 warm-up matmuls bridge the framework preamble to the first
    data chunk (~9.3us) so the PE clock (HAM K-level) ramps with no
    idle gap.
  - w0 chunks ride Sync (h-half 0) and Scalar (h-half 1) in k-order;
    xq chunks ride GpSimd. xf (gating input) and e1's weights are
    dep-gated AFTER w0 so they cannot steal the supply-critical
    bandwidth; e1's first half is split into k-chunks so its early
    k-groups don't wait on a 2MB-transfer-end semaphore.
  - Gating matmuls run right after phase B (absorbing its PSUM->acc
    copy drain); softmax transposes run after e1-mc0's first k-step.
  - e0's gate scale is applied in place by the ACT engine during e1;
    e1..e6 combine acc += gate_e * psum as one DVE stt from PSUM.
    Four PSUM-pool padding allocations keep the bank rotation aligned.
  - e7's LAST mc-group runs hc-major (k inner) so each hc tile's
    combine+output-DMA chases the matmul stream instead of serializing
    after it; the final hc is split in half so the last DMA chases the
    last half-stt.

Host-side prep (not device work): fp8 quantize (identical RNE cast the
reference performs), flat layout packing, bf16->f32 upcast of the
output and the final concat.
"""

import numpy as np
import ml_dtypes

import concourse.bacc as bacc
import concourse.bass as bass
import concourse.mybir as mybir
import concourse.tile as tile
from concourse import masks
from concourse.tile import add_dep_helper
from concourse.bass_utils import run_bass_kernel_spmd

M, D, H, E = 4096, 2048, 2048, 8
NCORES = 8
MS = M // NCORES          # tokens per core (512)
MC = MS // 128            # m-chunks of 128 partitions (4)
DS = D // 128             # d-subtiles of 128 (16)
KP = DS // 2              # DoubleRow d-pairs of 256 (8)
NH = 512                  # h columns per matmul (one PSUM bank of f32)
HC = H // NH              # h-chunks (4)
WJ = KP * 4               # wq dim-1 entries per expert (kp x hh x r)
N_WARM = 30               # dummy warm-up matmuls (N=128) for HAM ramp

_NC = None


def _build_program() -> bass.Bass:
    dt = mybir.dt
    nc = bacc.Bacc(None, target_bir_lowering=False)

    # Flat host-prearranged layouts (partition dim first, wide rows):
    #   xq/xf: [p, s, m] with d = s*128 + p
    #   wq:    [p, j, 1024] with j = ((e*KP + kp)*2 + hh)*2 + r,
    #          holding w^T[(2kp+r)*128 + p, hh*1024 + h']
    xq = nc.dram_tensor("xq", [128, DS, MS], dt.float8e4, kind="ExternalInput")
    xf = nc.dram_tensor("xf", [128, DS, MS], dt.bfloat16, kind="ExternalInput")
    wq = nc.dram_tensor("wq", [128, E * WJ, 1024], dt.float8e4,
                        kind="ExternalInput")
    gwt = nc.dram_tensor("gwt", [128, DS, E], dt.bfloat16, kind="ExternalInput")
    gb = nc.dram_tensor("gb", [E, 1], dt.float32, kind="ExternalInput")
    out = nc.dram_tensor("out", [MS, H], dt.bfloat16, kind="ExternalOutput")

    with tile.TileContext(nc) as tc, \
            tc.tile_pool(name="const", bufs=1) as constp, \
            tc.tile_pool(name="wpool", bufs=2) as wpool, \
            tc.tile_pool(name="outp", bufs=6) as outp, \
            tc.tile_pool(name="small", bufs=8) as small, \
            tc.tile_pool(name="ps", bufs=8, space="PSUM") as psp:

        # Persistent SBUF tensors.
        xq_sb = constp.tile([128, DS, MS], dt.float8e4, tag="xq")
        xf_sb = constp.tile([128, DS, MS], dt.bfloat16, tag="xf")
        gwt_sb = constp.tile([128, DS, E], dt.bfloat16, tag="gwt")
        gb_sb = constp.tile([E, 1], dt.float32, tag="gb")
        id8_sb = constp.tile([E, E], dt.float32, tag="id8")
        gate_sb = constp.tile([128, MC * E], dt.float32, tag="gate")
        lg_sb = constp.tile([E, MS], dt.float32, tag="lg")
        acc_sb = constp.tile([128, MC * H], dt.float32, tag="acc")
        warm_sb = constp.tile([128, 256], dt.bfloat16, tag="warm")

        masks.make_identity(nc, id8_sb[:])
        nc.gpsimd.memset(warm_sb[:], 0.25)

        # PE warm-up: keep the tensor engine busy from t~7.6us (end of
        # the framework preamble) until the first w0/xq chunks land
        # (~9.3us) so the HAM clock ramp never sees an idle gap.
        ps_warm = psp.tile([128, 128], dt.float32, tag="ps", name="ps_warm")
        for _ in range(N_WARM):
            nc.tensor.matmul(
                ps_warm[:], lhsT=warm_sb[:, 0:128], rhs=warm_sb[:, 128:256],
                start=True, stop=True,
            )

        # ---- DMA ladder ----
        # Each launcher engine (Sync/Scalar/GpSimd) feeds its own HW
        # DMA ring; a ring round-robins packets across ALL in-flight
        # transfers, so concurrent chunks finish together at the END.
        # Every ring is therefore dep-CHAINED (in-flight=1) so chunks
        # complete in consumption order at full ring rate (~4KB rows).
        # w0 streams k-major as full-kp 512KB chunks: even kp on Sync,
        # odd kp on Scalar, giving ~2 chunks per 3.3us against phase
        # A's 1.7us/kp consumption.
        w_sb0 = wpool.tile([128, WJ, 1024], dt.float8e4, tag="w")
        d_w0 = {}
        prev = {"sync": None, "scalar": None}
        for kp in range(KP):
            ring = "sync" if kp % 2 == 0 else "scalar"
            eng = nc.sync if kp % 2 == 0 else nc.scalar
            dj = eng.dma_start(
                w_sb0[:, 4 * kp:4 * kp + 4, :],
                wq[:, 4 * kp:4 * kp + 4, :])
            if prev[ring] is not None:
                add_dep_helper(dj.ins, prev[ring].ins,
                               reason=f"w0 {ring} ring chain")
            prev[ring] = dj
            d_w0[kp] = dj
        # xq: kp0 alone first (gates the very first matmul), the rest
        # as one wide transfer behind it on the GpSimd ring.
        d_xq0 = nc.gpsimd.dma_start(xq_sb[:, 0:2, :], xq[:, 0:2, :])
        d_xqr = nc.gpsimd.dma_start(xq_sb[:, 2:DS, :], xq[:, 2:DS, :])
        add_dep_helper(d_xqr.ins, d_xq0.ins, reason="xq ring chain")
        d_gwt = nc.gpsimd.dma_start(gwt_sb[:], gwt[:, :, :])
        add_dep_helper(d_gwt.ins, d_xqr.ins, reason="gpsimd ring chain")
        d_gb = nc.gpsimd.dma_start(gb_sb[:], gb[:, :])
        add_dep_helper(d_gb.ins, d_gwt.ins, reason="gpsimd ring chain")
        # xf (gating input): strictly after w0 so its ring cannot steal
        # shared-bus bandwidth from the supply-critical w0 stream.
        # Needed from ~31us (gating block); lands ~26us.
        d_xf = []
        dp = d_gb
        for j in range(2):
            dj = nc.gpsimd.dma_start(
                xf_sb[:, j * 8:(j + 1) * 8, :], xf[:, j * 8:(j + 1) * 8, :])
            add_dep_helper(dj.ins, dp.ins, reason="gpsimd ring chain")
            if j == 0:
                add_dep_helper(dj.ins, d_w0[KP - 2].ins,
                               reason="xf after w0 sync ring")
                add_dep_helper(dj.ins, d_w0[KP - 1].ins,
                               reason="xf after w0 scalar ring")
            d_xf.append(dj)
            dp = dj

        def rhs_ap(w_sb, kp, hc):
            j = 4 * kp + 2 * (hc // 2)
            q = hc % 2
            return w_sb[:, j:j + 2, q * 512:(q + 1) * 512]

        # Split PSUM->acc copies alternately across ACT and DVE so each
        # phase's copy chain drains twice as fast.
        def copy_out(i, dst, src):
            if i % 2 == 0:
                nc.scalar.copy(dst, src)
            else:
                nc.vector.tensor_copy(dst, src)

        # ---- Expert 0: m-split phases, k-major consumption ----
        # Phase A (mc0,1 x hc0-3) consumes w0 chunks as they arrive;
        # phase B (mc2,3) replays them from SBUF. PSUM -> acc UNSCALED.
        def e0_phase(mcs):
            pss = {
                mc: [psp.tile([128, NH], dt.float32, tag="ps",
                              name=f"ps0_{mc}_{hc}") for hc in range(HC)]
                for mc in mcs
            }
            for kp in range(KP):
                for mc in mcs:
                    lhsT = xq_sb[:, 2 * kp:2 * kp + 2, mc * 128:(mc + 1) * 128]
                    for hc in range(HC):
                        nc.tensor.matmul(
                            pss[mc][hc][:],
                            lhsT=lhsT,
                            rhs=rhs_ap(w_sb0, kp, hc),
                            start=(kp == 0),
                            stop=(kp == KP - 1),
                            perf_mode=mybir.MatmulPerfMode.DoubleRow,
                        )
            i = 0
            for mc in mcs:
                for hc in range(HC):
                    copy_out(i, acc_sb[:, mc * H + hc * NH:mc * H + (hc + 1) * NH],
                             pss[mc][hc][:])
                    i += 1

        e0_phase((0, 1))
        e0_phase((2, 3))

        # ---- Gating matmuls right after phase B (they absorb phase
        # B's copy-chain drain before e1's matmuls need those banks).
        ps_gt = psp.tile([E, MS], dt.float32, tag="ps", name="ps_gt")
        for s in range(DS):
            nc.tensor.matmul(
                ps_gt[:],
                lhsT=gwt_sb[:, s:s + 1, :],
                rhs=xf_sb[:, s:s + 1, :],
                start=(s == 0),
                stop=(s == DS - 1),
            )
        nc.vector.tensor_scalar_add(lg_sb[:], ps_gt[:], gb_sb[:])

        def emit_softmax():
            for mc in range(MC):
                pst = psp.tile([128, E], dt.float32, tag="ps", name=f"ps_t{mc}")
                nc.tensor.transpose(
                    pst[:], lg_sb[:, mc * 128:(mc + 1) * 128], id8_sb[:]
                )
                mx = small.tile([128, 1], dt.float32, tag="sm1")
                nc.vector.tensor_reduce(
                    mx[:], pst[:], mybir.AxisListType.X, mybir.AluOpType.max
                )
                nmx = small.tile([128, 1], dt.float32, tag="sm1")
                nc.vector.tensor_scalar_mul(nmx[:], mx[:], -1.0)
                ex = small.tile([128, E], dt.float32, tag="sm")
                ssum = small.tile([128, 1], dt.float32, tag="sm1")
                nc.scalar.activation(
                    ex[:], pst[:], mybir.ActivationFunctionType.Exp,
                    bias=nmx[:], scale=1.0, accum_out=ssum[:],
                )
                rcp = small.tile([128, 1], dt.float32, tag="sm1")
                nc.vector.reciprocal(rcp[:], ssum[:])
                nc.vector.tensor_scalar_mul(
                    gate_sb[:, mc * E:(mc + 1) * E], ex[:], rcp[:]
                )

        # ---- Experts 1..7: mc-major, DVE combine straight from PSUM ----
        # Output-launch queues: GpSimd only gets early tiles (its
        # end-of-kernel queue drain would otherwise serialize the
        # teardown behind a late transfer).
        out_q = {
            0: [nc.gpsimd, nc.gpsimd, nc.gpsimd, nc.gpsimd],
            1: [nc.scalar, nc.scalar, nc.scalar, nc.scalar],
            2: [nc.sync, nc.sync, nc.gpsimd, nc.scalar],
            3: [nc.scalar, nc.sync, nc.scalar, nc.sync],
        }
        sync_prev = d_w0[KP - 2]
        scalar_prev = d_w0[KP - 1]
        for e in range(1, E):
            w_sb = wpool.tile([128, WJ, 1024], dt.float8e4, tag="w")
            if e == 1:
                # e1's first half in k-chunks (so early k-groups gate on
                # 512KB, not a 2MB transfer-end semaphore).
                for kp in range(4):
                    dw = nc.sync.dma_start(
                        w_sb[:, 4 * kp:4 * kp + 4, :],
                        wq[:, (KP + kp) * 4:(KP + kp + 1) * 4, :])
                    add_dep_helper(dw.ins, sync_prev.ins,
                                   reason="sync ring chain")
                    sync_prev = dw
            else:
                dw = nc.sync.dma_start(
                    w_sb[:, 0:16, :],
                    wq[:, e * WJ:e * WJ + 16, :])
                add_dep_helper(dw.ins, sync_prev.ins, reason="sync ring chain")
                sync_prev = dw
            dw = nc.scalar.dma_start(
                w_sb[:, 16:32, :],
                wq[:, e * WJ + 16:(e + 1) * WJ, :])
            add_dep_helper(dw.ins, scalar_prev.ins, reason="scalar ring chain")
            scalar_prev = dw
            for mc in range(MC):
                if e == 1 and mc == 1:
                    # Rotation padding: the softmax block inserted 5
                    # PSUM allocations (ps_gt + 4 transposes), breaking
                    # the 4-slot alternation between mc-groups. Four
                    # pad slots (with DVE memsets emitted AFTER mc0's
                    # combines, so the FIFO has no cycle) realign the
                    # ring: every matmul group again lands on banks
                    # freed a full window earlier.
                    for p in range(4):
                        pad = psp.tile([128, 1], dt.float32, tag="ps",
                                       name=f"ps_pad{p}")
                        nc.vector.memset(pad[:], 0.0)
                msl = slice(mc * 128, (mc + 1) * 128)
                pss = [
                    psp.tile([128, NH], dt.float32, tag="ps", name=f"ps_{e}_{mc}_{i}")
                    for i in range(HC)
                ]
                g_ap = gate_sb[:, mc * E + e:mc * E + e + 1]
                if e == E - 1 and mc == MC - 1:
                    # Final group hc-major: each hc tile's combine+DMA
                    # chases the matmul stream; only the last half-tile
                    # trails the last matmul.
                    for hc in range(HC):
                        for k in range(KP):
                            nc.tensor.matmul(
                                pss[hc][:],
                                lhsT=xq_sb[:, 2 * k:2 * k + 2, msl],
                                rhs=rhs_ap(w_sb, k, hc),
                                start=(k == 0),
                                stop=(k == KP - 1),
                                perf_mode=mybir.MatmulPerfMode.DoubleRow,
                            )
                        a_ap = acc_sb[:, mc * H + hc * NH:mc * H + (hc + 1) * NH]
                        if hc < HC - 1:
                            ot = outp.tile([128, NH], dt.bfloat16, tag="ot")
                            nc.vector.scalar_tensor_tensor(
                                ot[:], pss[hc][:], g_ap, a_ap,
                                op0=mybir.AluOpType.mult,
                                op1=mybir.AluOpType.add,
                            )
                            out_q[mc][hc].dma_start(
                                out[msl, hc * NH:(hc + 1) * NH], ot[:]
                            )
                        else:
                            ot = outp.tile([128, NH], dt.bfloat16, tag="ot")
                            for half, q in ((0, nc.scalar), (1, nc.sync)):
                                csl = slice(half * 256, (half + 1) * 256)
                                nc.vector.scalar_tensor_tensor(
                                    ot[:, csl], pss[hc][:, csl], g_ap,
                                    a_ap[:, csl],
                                    op0=mybir.AluOpType.mult,
                                    op1=mybir.AluOpType.add,
                                )
                                q.dma_start(
                                    out[msl, hc * NH + half * 256:
                                        hc * NH + (half + 1) * 256],
                                    ot[:, csl],
                                )
                    continue
                for k in range(KP):
                    lhsT = xq_sb[:, 2 * k:2 * k + 2, msl]
                    for hc in range(HC):
                        nc.tensor.matmul(
                            pss[hc][:],
                            lhsT=lhsT,
                            rhs=rhs_ap(w_sb, k, hc),
                            start=(k == 0),
                            stop=(k == KP - 1),
                            perf_mode=mybir.MatmulPerfMode.DoubleRow,
                        )
                    if e == 1 and mc == 0 and k == 0:
                        # Softmax transposes here: the PE is one k-step
                        # into e1, lg_sb is ready, phase B's copies are
                        # drained — no PE wait.
                        emit_softmax()
                if e == 1:
                    # Deferred e0 gate scale, on ACT (activation Copy
                    # with per-partition scale) so the DVE stays free
                    # for the combines.
                    g0_ap = gate_sb[:, mc * E:mc * E + 1]
                    for hc in range(HC):
                        a_ap = acc_sb[:, mc * H + hc * NH:mc * H + (hc + 1) * NH]
                        nc.scalar.activation(
                            a_ap, a_ap, mybir.ActivationFunctionType.Copy,
                            scale=g0_ap,
                        )
                for hc in range(HC):
                    a_ap = acc_sb[:, mc * H + hc * NH:mc * H + (hc + 1) * NH]
                    if e < E - 1:
                        nc.vector.scalar_tensor_tensor(
                            a_ap, pss[hc][:], g_ap, a_ap,
                            op0=mybir.AluOpType.mult, op1=mybir.AluOpType.add,
                        )
                    else:
                        ot = outp.tile([128, NH], dt.bfloat16, tag="ot")
                        nc.vector.scalar_tensor_tensor(
                            ot[:], pss[hc][:], g_ap, a_ap,
                            op0=mybir.AluOpType.mult, op1=mybir.AluOpType.add,
                        )
                        out_q[mc][hc].dma_start(
                            out[msl, hc * NH:(hc + 1) * NH], ot[:]
                        )

    nc.compile()
    return nc


def _get_nc() -> bass.Bass:
    global _NC
    if _NC is None:
        _NC = _build_program()
    return _NC


def _prep_in_maps(x, gate_w, gate_b, expert_w):
    f8fn = ml_dtypes.float8_e4m3fn
    f8trn = ml_dtypes.float8_e4m3  # same bits as e4m3fn for |v| <= 240

    x = np.asarray(x, dtype=np.float32)
    gate_w = np.asarray(gate_w, dtype=np.float32)
    gate_b = np.asarray(gate_b, dtype=np.float32)
    expert_w = np.asarray(expert_w, dtype=np.float32)

    # x^T: [D, M]; quantized and bf16 (gating) copies.
    xT = np.ascontiguousarray(x.T)                       # [D, M] f32
    xT_bf = xT.astype(ml_dtypes.bfloat16)                # [D, M] bf16 (gating)
    xqT = xT.astype(f8fn).view(f8trn)                    # [D, M] fp8
    # expert_w [E, H, D] -> w^T per expert [E, D, H], quantized, packed
    # into the flat [128, j, 1024] device layout with
    # j = ((e*KP + kp)*2 + hh)*2 + r and d = (2*kp + r)*128 + p.
    wqT = np.ascontiguousarray(
        expert_w.transpose(0, 2, 1)
    ).astype(f8fn).view(f8trn)                           # [E, D, H]
    wq_flat = np.ascontiguousarray(
        wqT.reshape(E, KP, 2, 128, 2, 1024)
           .transpose(3, 0, 1, 4, 2, 5)
           .reshape(128, E * WJ, 1024)
    )
    gwt_flat = np.ascontiguousarray(
        gate_w.T.astype(ml_dtypes.bfloat16)
              .reshape(DS, 128, E).transpose(1, 0, 2)
    )
    gbb = np.ascontiguousarray(gate_b.reshape(E, 1))

    in_maps = []
    for c in range(NCORES):
        csl = slice(c * MS, (c + 1) * MS)
        xq_c = np.ascontiguousarray(
            xqT[:, csl].reshape(DS, 128, MS).transpose(1, 0, 2))
        xf_c = np.ascontiguousarray(
            xT_bf[:, csl].reshape(DS, 128, MS).transpose(1, 0, 2))
        in_maps.append({
            "xq": xq_c,
            "xf": xf_c,
            "wq": wq_flat,
            "gwt": gwt_flat,
            "gb": gbb,
        })
    return in_maps


def kernel(x, gate_w, gate_b, expert_w, _trace=False, _trace_kwargs=None):
    nc = _get_nc()
    in_maps = _prep_in_maps(x, gate_w, gate_b, expert_w)
    kw = {}
    if _trace:
        kw["trace"] = True
        kw.update(_trace_kwargs or {})
    res = run_bass_kernel_spmd(nc, in_maps, core_ids=list(range(NCORES)), **kw)
    outp = np.concatenate(
        [np.asarray(res.results[c]["out"]).astype(np.float32)
         for c in range(NCORES)],
        axis=0,
    )
    if _trace:
        return outp, res
    return outp


# revision 12
# speedup vs baseline: 1.0120x; 1.0120x over previous
"""MoE-with-DeepGEMM kernel for 8 Trainium2 NeuronCores.

Problem: M=4096 tokens, D=2048 in-dim, H=2048 out-dim, E=8 experts.
    gate = softmax(x @ gate_w.T + gate_b)            # [M, E], fp32
    y    = (q8(x) @ q8(expert_w[e]).T) -> bf16       # [E, M, H]
    out  = sum_e gate[:, e, None] * y[e].astype(f32) # [M, H]

Strategy: data-parallel over tokens (M). Each of the 8 cores gets
M/8 = 512 tokens, all 8 experts' weights, and computes its output slice
independently — no collectives; the host concatenates the slices.

The PE issue rate is the wall: 1024 DoubleRow matmuls x ~213 ns plus 16
gating matmuls. The schedule keeps the PE issuing back-to-back and the
HBM supply matched to consumption order:
  - All inputs are HOST-PREARRANGED into flat per-partition layouts so
    every DMA descriptor row is wide (2KB+) and runs at full HBM rate
    (~350 GB/s); the baseline's rearranged transfers with 512B-1KB rows
    only reached ~200-250 GB/s in the critical first 20us.
  - e0 is restructured into m-split phases: phase A (mc0,1 x all hc)
    consumes w0 strictly k-major in 256KB chunks AS THEY ARRIVE;
    phase B (mc2,3) reuses w0 entirely from SBUF (zero DMA). The
    baseline's h-split phases interleaved consumption against arrival
    and stalled the PE mid-stream (HAM re-throttle cost ~4us).
  - 16 bf16 warm-up matmuls bridge the framework preamble to the first
    data chunk (~9.3us) so the PE clock (HAM K-level) ramps with no
    idle gap.
  - w0 chunks ride Sync (h-half 0) and Scalar (h-half 1) in k-order;
    xq chunks ride GpSimd. xf (gating input) and e1's weights are
    dep-gated AFTER w0 so they cannot steal the supply-critical
    bandwidth; e1's first half is split into k-chunks so its early
    k-groups don't wait on a 2MB-transfer-end semaphore.
  - Gating matmuls run right after phase B (absorbing its PSUM->acc
    copy drain); softmax transposes run after e1-mc0's first k-step.
  - e0's gate scale is applied in place by the ACT engine during e1;
    e1..e6 combine acc += gate_e * psum as one DVE stt from PSUM.
    Four PSUM-pool padding allocations keep the bank rotation aligned.
  - e7's LAST mc-group runs hc-major (k inner) so each hc tile's
    combine+output-DMA chases the matmul stream instead of serializing
    after it; the final hc is split in half so the last DMA chases the
    last half-stt.

Host-side prep (not device work): fp8 quantize (identical RNE cast the
reference performs), flat layout packing, bf16->f32 upcast of the
output and the final concat.
"""

import numpy as np
import ml_dtypes

import concourse.bacc as bacc
import concourse.bass as bass
import concourse.mybir as mybir
import concourse.tile as tile
from concourse import masks
from concourse.tile import add_dep_helper
from concourse.bass_utils import run_bass_kernel_spmd

M, D, H, E = 4096, 2048, 2048, 8
NCORES = 8
MS = M // NCORES          # tokens per core (512)
MC = MS // 128            # m-chunks of 128 partitions (4)
DS = D // 128             # d-subtiles of 128 (16)
KP = DS // 2              # DoubleRow d-pairs of 256 (8)
NH = 512                  # h columns per matmul (one PSUM bank of f32)
HC = H // NH              # h-chunks (4)
WJ = KP * 4               # wq dim-1 entries per expert (kp x hh x r)
N_WARM = 42               # dummy warm-up matmuls (N=128) for HAM ramp

_NC = None


def _build_program() -> bass.Bass:
    dt = mybir.dt
    nc = bacc.Bacc(None, target_bir_lowering=False)

    # Flat host-prearranged layouts (partition dim first, wide rows):
    #   xq/xf: [p, s, m] with d = s*128 + p
    #   wq:    [p, j, 1024] with j = ((e*KP + kp)*2 + hh)*2 + r,
    #          holding w^T[(2kp+r)*128 + p, hh*1024 + h']
    xq = nc.dram_tensor("xq", [128, DS, MS], dt.float8e4, kind="ExternalInput")
    xf = nc.dram_tensor("xf", [128, DS, MS], dt.bfloat16, kind="ExternalInput")
    wq = nc.dram_tensor("wq", [128, E * WJ, 1024], dt.float8e4,
                        kind="ExternalInput")
    gwt = nc.dram_tensor("gwt", [128, DS, E], dt.bfloat16, kind="ExternalInput")
    gb = nc.dram_tensor("gb", [E, 1], dt.float32, kind="ExternalInput")
    out = nc.dram_tensor("out", [MS, H], dt.bfloat16, kind="ExternalOutput")

    with tile.TileContext(nc) as tc, \
            tc.tile_pool(name="const", bufs=1) as constp, \
            tc.tile_pool(name="wpool", bufs=2) as wpool, \
            tc.tile_pool(name="outp", bufs=6) as outp, \
            tc.tile_pool(name="small", bufs=8) as small, \
            tc.tile_pool(name="ps", bufs=8, space="PSUM") as psp:

        # Persistent SBUF tensors.
        xq_sb = constp.tile([128, DS, MS], dt.float8e4, tag="xq")
        xf_sb = constp.tile([128, DS, MS], dt.bfloat16, tag="xf")
        gwt_sb = constp.tile([128, DS, E], dt.bfloat16, tag="gwt")
        gb_sb = constp.tile([E, 1], dt.float32, tag="gb")
        id8_sb = constp.tile([E, E], dt.float32, tag="id8")
        gate_sb = constp.tile([128, MC * E], dt.float32, tag="gate")
        lg_sb = constp.tile([E, MS], dt.float32, tag="lg")
        acc_sb = constp.tile([128, MC * H], dt.float32, tag="acc")
        warm_sb = constp.tile([128, 256], dt.bfloat16, tag="warm")

        masks.make_identity(nc, id8_sb[:])
        nc.gpsimd.memset(warm_sb[:], 0.25)

        # PE warm-up: keep the tensor engine busy from t~7.6us (end of
        # the framework preamble) until the first w0/xq chunks land
        # (~9.3us) so the HAM clock ramp never sees an idle gap.
        ps_warm = psp.tile([128, 128], dt.float32, tag="ps", name="ps_warm")
        for _ in range(N_WARM):
            nc.tensor.matmul(
                ps_warm[:], lhsT=warm_sb[:, 0:128], rhs=warm_sb[:, 128:256],
                start=True, stop=True,
            )

        # ---- DMA ladder ----
        # Each launcher engine (Sync/Scalar/GpSimd) feeds its own HW
        # DMA ring; a ring round-robins packets across ALL in-flight
        # transfers, so concurrent chunks finish together at the END.
        # Every ring is therefore dep-CHAINED (in-flight=1) so chunks
        # complete in consumption order at full ring rate (~4KB rows).
        # w0 streams k-major as full-kp 512KB chunks: even kp on Sync,
        # odd kp on Scalar, giving ~2 chunks per 3.3us against phase
        # A's 1.7us/kp consumption.
        # xq: ONE wide transfer (8KB rows) heading the GpSimd ring —
        # narrow (1KB-row) xq chunks measured only 18-48 GB/s and
        # starved phase A's lhsT.
        d_xq = nc.gpsimd.dma_start(xq_sb[:, :, :], xq[:, :, :])
        w_sb0 = wpool.tile([128, WJ, 1024], dt.float8e4, tag="w")
        d_w0 = {}
        ring_eng = [nc.sync, nc.scalar, nc.gpsimd]
        ring_names = ["sync", "scalar", "gpsimd"]
        prev = {0: None, 1: None, 2: d_xq}
        for kp in range(KP):
            r = kp % 3
            dj = ring_eng[r].dma_start(
                w_sb0[:, 4 * kp:4 * kp + 4, :],
                wq[:, 4 * kp:4 * kp + 4, :])
            if prev[r] is not None:
                add_dep_helper(dj.ins, prev[r].ins,
                               reason=f"w0 {ring_names[r]} ring chain")
            prev[r] = dj
            d_w0[kp] = dj
        d_gwt = nc.gpsimd.dma_start(gwt_sb[:], gwt[:, :, :])
        add_dep_helper(d_gwt.ins, prev[2].ins, reason="gpsimd ring chain")
        d_gb = nc.gpsimd.dma_start(gb_sb[:], gb[:, :])
        add_dep_helper(d_gb.ins, d_gwt.ins, reason="gpsimd ring chain")
        # xf (gating input): strictly after w0 so its ring cannot steal
        # shared-bus bandwidth from the supply-critical w0 stream.
        # Needed from ~31us (gating block); lands ~26us.
        d_xf = []
        dp = d_gb
        for j in range(2):
            dj = nc.gpsimd.dma_start(
                xf_sb[:, j * 8:(j + 1) * 8, :], xf[:, j * 8:(j + 1) * 8, :])
            add_dep_helper(dj.ins, dp.ins, reason="gpsimd ring chain")
            if j == 0:
                add_dep_helper(dj.ins, d_w0[6].ins,
                               reason="xf after w0 sync ring")
                add_dep_helper(dj.ins, d_w0[7].ins,
                               reason="xf after w0 scalar ring")
            d_xf.append(dj)
            dp = dj

        def rhs_ap(w_sb, kp, hc):
            j = 4 * kp + 2 * (hc // 2)
            q = hc % 2
            return w_sb[:, j:j + 2, q * 512:(q + 1) * 512]

        # Split PSUM->acc copies alternately across ACT and DVE so each
        # phase's copy chain drains twice as fast.
        def copy_out(i, dst, src):
            if i % 2 == 0:
                nc.scalar.copy(dst, src)
            else:
                nc.vector.tensor_copy(dst, src)

        # ---- Expert 0: m-split phases, k-major consumption ----
        # Phase A (mc0,1 x hc0-3) consumes w0 chunks as they arrive;
        # phase B (mc2,3) replays them from SBUF. PSUM -> acc UNSCALED.
        def e0_phase(mcs):
            pss = {
                mc: [psp.tile([128, NH], dt.float32, tag="ps",
                              name=f"ps0_{mc}_{hc}") for hc in range(HC)]
                for mc in mcs
            }
            for kp in range(KP):
                for mc in mcs:
                    lhsT = xq_sb[:, 2 * kp:2 * kp + 2, mc * 128:(mc + 1) * 128]
                    for hc in range(HC):
                        nc.tensor.matmul(
                            pss[mc][hc][:],
                            lhsT=lhsT,
                            rhs=rhs_ap(w_sb0, kp, hc),
                            start=(kp == 0),
                            stop=(kp == KP - 1),
                            perf_mode=mybir.MatmulPerfMode.DoubleRow,
                        )
            i = 0
            for mc in mcs:
                for hc in range(HC):
                    copy_out(i, acc_sb[:, mc * H + hc * NH:mc * H + (hc + 1) * NH],
                             pss[mc][hc][:])
                    i += 1

        e0_phase((0, 1))
        e0_phase((2, 3))

        # ---- Gating matmuls right after phase B (they absorb phase
        # B's copy-chain drain before e1's matmuls need those banks).
        ps_gt = psp.tile([E, MS], dt.float32, tag="ps", name="ps_gt")
        for s in range(DS):
            nc.tensor.matmul(
                ps_gt[:],
                lhsT=gwt_sb[:, s:s + 1, :],
                rhs=xf_sb[:, s:s + 1, :],
                start=(s == 0),
                stop=(s == DS - 1),
            )
        nc.vector.tensor_scalar_add(lg_sb[:], ps_gt[:], gb_sb[:])

        def emit_softmax():
            for mc in range(MC):
                pst = psp.tile([128, E], dt.float32, tag="ps", name=f"ps_t{mc}")
                nc.tensor.transpose(
                    pst[:], lg_sb[:, mc * 128:(mc + 1) * 128], id8_sb[:]
                )
                mx = small.tile([128, 1], dt.float32, tag="sm1")
                nc.vector.tensor_reduce(
                    mx[:], pst[:], mybir.AxisListType.X, mybir.AluOpType.max
                )
                nmx = small.tile([128, 1], dt.float32, tag="sm1")
                nc.vector.tensor_scalar_mul(nmx[:], mx[:], -1.0)
                ex = small.tile([128, E], dt.float32, tag="sm")
                ssum = small.tile([128, 1], dt.float32, tag="sm1")
                nc.scalar.activation(
                    ex[:], pst[:], mybir.ActivationFunctionType.Exp,
                    bias=nmx[:], scale=1.0, accum_out=ssum[:],
                )
                rcp = small.tile([128, 1], dt.float32, tag="sm1")
                nc.vector.reciprocal(rcp[:], ssum[:])
                nc.vector.tensor_scalar_mul(
                    gate_sb[:, mc * E:(mc + 1) * E], ex[:], rcp[:]
                )

        # ---- Experts 1..7: mc-major, DVE combine straight from PSUM ----
        # Output-launch queues: GpSimd only gets early tiles (its
        # end-of-kernel queue drain would otherwise serialize the
        # teardown behind a late transfer).
        out_q = {
            0: [nc.gpsimd, nc.gpsimd, nc.gpsimd, nc.gpsimd],
            1: [nc.scalar, nc.scalar, nc.scalar, nc.scalar],
            2: [nc.sync, nc.sync, nc.gpsimd, nc.scalar],
            3: [nc.scalar, nc.sync, nc.scalar, nc.sync],
        }
        sync_prev = d_w0[6]
        scalar_prev = d_w0[7]
        for e in range(1, E):
            w_sb = wpool.tile([128, WJ, 1024], dt.float8e4, tag="w")
            if e == 1:
                # e1's first half in k-chunks (so early k-groups gate on
                # 512KB, not a 2MB transfer-end semaphore).
                for kp in range(4):
                    dw = nc.sync.dma_start(
                        w_sb[:, 4 * kp:4 * kp + 4, :],
                        wq[:, (KP + kp) * 4:(KP + kp + 1) * 4, :])
                    add_dep_helper(dw.ins, sync_prev.ins,
                                   reason="sync ring chain")
                    sync_prev = dw
            else:
                dw = nc.sync.dma_start(
                    w_sb[:, 0:16, :],
                    wq[:, e * WJ:e * WJ + 16, :])
                add_dep_helper(dw.ins, sync_prev.ins, reason="sync ring chain")
                sync_prev = dw
            dw = nc.scalar.dma_start(
                w_sb[:, 16:32, :],
                wq[:, e * WJ + 16:(e + 1) * WJ, :])
            add_dep_helper(dw.ins, scalar_prev.ins, reason="scalar ring chain")
            scalar_prev = dw
            for mc in range(MC):
                if e == 1 and mc == 1:
                    # Rotation padding: the softmax block inserted 5
                    # PSUM allocations (ps_gt + 4 transposes), breaking
                    # the 4-slot alternation between mc-groups. Four
                    # pad slots (with DVE memsets emitted AFTER mc0's
                    # combines, so the FIFO has no cycle) realign the
                    # ring: every matmul group again lands on banks
                    # freed a full window earlier.
                    for p in range(4):
                        pad = psp.tile([128, 1], dt.float32, tag="ps",
                                       name=f"ps_pad{p}")
                        nc.vector.memset(pad[:], 0.0)
                msl = slice(mc * 128, (mc + 1) * 128)
                pss = [
                    psp.tile([128, NH], dt.float32, tag="ps", name=f"ps_{e}_{mc}_{i}")
                    for i in range(HC)
                ]
                g_ap = gate_sb[:, mc * E + e:mc * E + e + 1]
                if e == E - 1 and mc == MC - 1:
                    # Final group hc-major: each hc tile's combine+DMA
                    # chases the matmul stream; only the last half-tile
                    # trails the last matmul.
                    for hc in range(HC):
                        for k in range(KP):
                            nc.tensor.matmul(
                                pss[hc][:],
                                lhsT=xq_sb[:, 2 * k:2 * k + 2, msl],
                                rhs=rhs_ap(w_sb, k, hc),
                                start=(k == 0),
                                stop=(k == KP - 1),
                                perf_mode=mybir.MatmulPerfMode.DoubleRow,
                            )
                        a_ap = acc_sb[:, mc * H + hc * NH:mc * H + (hc + 1) * NH]
                        if hc < HC - 1:
                            ot = outp.tile([128, NH], dt.bfloat16, tag="ot")
                            nc.vector.scalar_tensor_tensor(
                                ot[:], pss[hc][:], g_ap, a_ap,
                                op0=mybir.AluOpType.mult,
                                op1=mybir.AluOpType.add,
                            )
                            out_q[mc][hc].dma_start(
                                out[msl, hc * NH:(hc + 1) * NH], ot[:]
                            )
                        else:
                            ot = outp.tile([128, NH], dt.bfloat16, tag="ot")
                            for half, q in ((0, nc.scalar), (1, nc.sync)):
                                csl = slice(half * 256, (half + 1) * 256)
                                nc.vector.scalar_tensor_tensor(
                                    ot[:, csl], pss[hc][:, csl], g_ap,
                                    a_ap[:, csl],
                                    op0=mybir.AluOpType.mult,
                                    op1=mybir.AluOpType.add,
                                )
                                q.dma_start(
                                    out[msl, hc * NH + half * 256:
                                        hc * NH + (half + 1) * 256],
                                    ot[:, csl],
                                )
                    continue
                for k in range(KP):
                    lhsT = xq_sb[:, 2 * k:2 * k + 2, msl]
                    for hc in range(HC):
                        nc.tensor.matmul(
                            pss[hc][:],
                            lhsT=lhsT,
                            rhs=rhs_ap(w_sb, k, hc),
                            start=(k == 0),
                            stop=(k == KP - 1),
                            perf_mode=mybir.MatmulPerfMode.DoubleRow,
                        )
                    if e == 1 and mc == 0 and k == 0:
                        # Softmax transposes here: the PE is one k-step
                        # into e1, lg_sb is ready, phase B's copies are
                        # drained — no PE wait.
                        emit_softmax()
                if e == 1:
                    # Deferred e0 gate scale, on ACT (activation Copy
                    # with per-partition scale) so the DVE stays free
                    # for the combines.
                    g0_ap = gate_sb[:, mc * E:mc * E + 1]
                    for hc in range(HC):
                        a_ap = acc_sb[:, mc * H + hc * NH:mc * H + (hc + 1) * NH]
                        nc.scalar.activation(
                            a_ap, a_ap, mybir.ActivationFunctionType.Copy,
                            scale=g0_ap,
                        )
                for hc in range(HC):
                    a_ap = acc_sb[:, mc * H + hc * NH:mc * H + (hc + 1) * NH]
                    if e < E - 1:
                        nc.vector.scalar_tensor_tensor(
                            a_ap, pss[hc][:], g_ap, a_ap,
                            op0=mybir.AluOpType.mult, op1=mybir.AluOpType.add,
                        )
                    else:
                        ot = outp.tile([128, NH], dt.bfloat16, tag="ot")
                        nc.vector.scalar_tensor_tensor(
                            ot[:], pss[hc][:], g_ap, a_ap,
                            op0=mybir.AluOpType.mult, op1=mybir.AluOpType.add,
                        )
                        out_q[mc][hc].dma_start(
                            out[msl, hc * NH:(hc + 1) * NH], ot[:]
                        )

    nc.compile()
    return nc


def _get_nc() -> bass.Bass:
    global _NC
    if _NC is None:
        _NC = _build_program()
    return _NC


def _prep_in_maps(x, gate_w, gate_b, expert_w):
    f8fn = ml_dtypes.float8_e4m3fn
    f8trn = ml_dtypes.float8_e4m3  # same bits as e4m3fn for |v| <= 240

    x = np.asarray(x, dtype=np.float32)
    gate_w = np.asarray(gate_w, dtype=np.float32)
    gate_b = np.asarray(gate_b, dtype=np.float32)
    expert_w = np.asarray(expert_w, dtype=np.float32)

    # x^T: [D, M]; quantized and bf16 (gating) copies.
    xT = np.ascontiguousarray(x.T)                       # [D, M] f32
    xT_bf = xT.astype(ml_dtypes.bfloat16)                # [D, M] bf16 (gating)
    xqT = xT.astype(f8fn).view(f8trn)                    # [D, M] fp8
    # expert_w [E, H, D] -> w^T per expert [E, D, H], quantized, packed
    # into the flat [128, j, 1024] device layout with
    # j = ((e*KP + kp)*2 + hh)*2 + r and d = (2*kp + r)*128 + p.
    wqT = np.ascontiguousarray(
        expert_w.transpose(0, 2, 1)
    ).astype(f8fn).view(f8trn)                           # [E, D, H]
    wq_flat = np.ascontiguousarray(
        wqT.reshape(E, KP, 2, 128, 2, 1024)
           .transpose(3, 0, 1, 4, 2, 5)
           .reshape(128, E * WJ, 1024)
    )
    gwt_flat = np.ascontiguousarray(
        gate_w.T.astype(ml_dtypes.bfloat16)
              .reshape(DS, 128, E).transpose(1, 0, 2)
    )
    gbb = np.ascontiguousarray(gate_b.reshape(E, 1))

    in_maps = []
    for c in range(NCORES):
        csl = slice(c * MS, (c + 1) * MS)
        xq_c = np.ascontiguousarray(
            xqT[:, csl].reshape(DS, 128, MS).transpose(1, 0, 2))
        xf_c = np.ascontiguousarray(
            xT_bf[:, csl].reshape(DS, 128, MS).transpose(1, 0, 2))
        in_maps.append({
            "xq": xq_c,
            "xf": xf_c,
            "wq": wq_flat,
            "gwt": gwt_flat,
            "gb": gbb,
        })
    return in_maps


def kernel(x, gate_w, gate_b, expert_w, _trace=False, _trace_kwargs=None):
    nc = _get_nc()
    in_maps = _prep_in_maps(x, gate_w, gate_b, expert_w)
    kw = {}
    if _trace:
        kw["trace"] = True
        kw.update(_trace_kwargs or {})
    res = run_bass_kernel_spmd(nc, in_maps, core_ids=list(range(NCORES)), **kw)
    outp = np.concatenate(
        [np.asarray(res.results[c]["out"]).astype(np.float32)
         for c in range(NCORES)],
        axis=0,
    )
    if _trace:
        return outp, res
    return outp


# revision 15
# speedup vs baseline: 1.0195x; 1.0074x over previous
"""MoE-with-DeepGEMM kernel for 8 Trainium2 NeuronCores.

Problem: M=4096 tokens, D=2048 in-dim, H=2048 out-dim, E=8 experts.
    gate = softmax(x @ gate_w.T + gate_b)            # [M, E], fp32
    y    = (q8(x) @ q8(expert_w[e]).T) -> bf16       # [E, M, H]
    out  = sum_e gate[:, e, None] * y[e].astype(f32) # [M, H]

Strategy: data-parallel over tokens (M). Each of the 8 cores gets
M/8 = 512 tokens, all 8 experts' weights, and computes its output slice
independently — no collectives; the host concatenates the slices.

The PE issue rate is the wall: 1024 DoubleRow matmuls x ~213 ns plus 16
gating matmuls. The schedule keeps the PE issuing back-to-back and the
HBM supply matched to consumption order:
  - All inputs are HOST-PREARRANGED into flat per-partition layouts so
    every DMA descriptor row is wide (2KB+) and runs at full HBM rate
    (~350 GB/s); the baseline's rearranged transfers with 512B-1KB rows
    only reached ~200-250 GB/s in the critical first 20us.
  - e0 is restructured into m-split phases: phase A (mc0,1 x all hc)
    consumes w0 strictly k-major in 256KB chunks AS THEY ARRIVE;
    phase B (mc2,3) reuses w0 entirely from SBUF (zero DMA). The
    baseline's h-split phases interleaved consumption against arrival
    and stalled the PE mid-stream (HAM re-throttle cost ~4us).
  - 16 bf16 warm-up matmuls bridge the framework preamble to the first
    data chunk (~9.3us) so the PE clock (HAM K-level) ramps with no
    idle gap.
  - w0 chunks ride Sync (h-half 0) and Scalar (h-half 1) in k-order;
    xq chunks ride GpSimd. xf (gating input) and e1's weights are
    dep-gated AFTER w0 so they cannot steal the supply-critical
    bandwidth; e1's first half is split into k-chunks so its early
    k-groups don't wait on a 2MB-transfer-end semaphore.
  - Gating matmuls run right after phase B (absorbing its PSUM->acc
    copy drain); softmax transposes run after e1-mc0's first k-step.
  - e0's gate scale is applied in place by the ACT engine during e1;
    e1..e6 combine acc += gate_e * psum as one DVE stt from PSUM.
    Four PSUM-pool padding allocations keep the bank rotation aligned.
  - e7's LAST mc-group runs hc-major (k inner) so each hc tile's
    combine+output-DMA chases the matmul stream instead of serializing
    after it; the final hc is split in half so the last DMA chases the
    last half-stt.

Host-side prep (not device work): fp8 quantize (identical RNE cast the
reference performs), flat layout packing, bf16->f32 upcast of the
output and the final concat.
"""

import numpy as np
import ml_dtypes

import concourse.bacc as bacc
import concourse.bass as bass
import concourse.mybir as mybir
import concourse.tile as tile
from concourse import masks
from concourse.tile import add_dep_helper
from concourse.bass_utils import run_bass_kernel_spmd

M, D, H, E = 4096, 2048, 2048, 8
NCORES = 8
MS = M // NCORES          # tokens per core (512)
MC = MS // 128            # m-chunks of 128 partitions (4)
DS = D // 128             # d-subtiles of 128 (16)
KP = DS // 2              # DoubleRow d-pairs of 256 (8)
NH = 512                  # h columns per matmul (one PSUM bank of f32)
HC = H // NH              # h-chunks (4)
WJ = KP * 4               # wq dim-1 entries per expert (kp x hh x r)
N_WARM = 48               # dummy warm-up matmuls (N=128) for HAM ramp

_NC = None


def _build_program() -> bass.Bass:
    dt = mybir.dt
    nc = bacc.Bacc(None, target_bir_lowering=False)

    # Flat host-prearranged layouts (partition dim first, wide rows):
    #   xq/xf: [p, s, m] with d = s*128 + p
    #   wq:    [p, j, 1024] with j = ((e*KP + kp)*2 + hh)*2 + r,
    #          holding w^T[(2kp+r)*128 + p, hh*1024 + h']
    xq = nc.dram_tensor("xq", [128, DS, MS], dt.float8e4, kind="ExternalInput")
    xf = nc.dram_tensor("xf", [128, DS, MS], dt.bfloat16, kind="ExternalInput")
    wq = nc.dram_tensor("wq", [128, E * WJ, 1024], dt.float8e4,
                        kind="ExternalInput")
    gwt = nc.dram_tensor("gwt", [128, DS, E], dt.bfloat16, kind="ExternalInput")
    gb = nc.dram_tensor("gb", [E, 1], dt.float32, kind="ExternalInput")
    out = nc.dram_tensor("out", [MS, H], dt.bfloat16, kind="ExternalOutput")

    with tile.TileContext(nc) as tc, \
            tc.tile_pool(name="const", bufs=1) as constp, \
            tc.tile_pool(name="wpool", bufs=2) as wpool, \
            tc.tile_pool(name="outp", bufs=6) as outp, \
            tc.tile_pool(name="small", bufs=8) as small, \
            tc.tile_pool(name="ps", bufs=8, space="PSUM") as psp:

        # Persistent SBUF tensors.
        xq_sb = constp.tile([128, DS, MS], dt.float8e4, tag="xq")
        xf_sb = constp.tile([128, DS, MS], dt.bfloat16, tag="xf")
        gwt_sb = constp.tile([128, DS, E], dt.bfloat16, tag="gwt")
        gb_sb = constp.tile([E, 1], dt.float32, tag="gb")
        id8_sb = constp.tile([E, E], dt.float32, tag="id8")
        gate_sb = constp.tile([128, MC * E], dt.float32, tag="gate")
        lg_sb = constp.tile([E, MS], dt.float32, tag="lg")
        acc_sb = constp.tile([128, MC * H], dt.float32, tag="acc")
        warm_sb = constp.tile([128, 256], dt.bfloat16, tag="warm")

        masks.make_identity(nc, id8_sb[:])
        nc.gpsimd.memset(warm_sb[:], 0.25)

        # PE warm-up: keep the tensor engine busy from t~7.6us (end of
        # the framework preamble) until the first w0/xq chunks land
        # (~9.3us) so the HAM clock ramp never sees an idle gap.
        ps_warm = psp.tile([128, 128], dt.float32, tag="ps", name="ps_warm")
        for _ in range(N_WARM):
            nc.tensor.matmul(
                ps_warm[:], lhsT=warm_sb[:, 0:128], rhs=warm_sb[:, 128:256],
                start=True, stop=True,
            )

        # ---- DMA ladder ----
        # Each launcher engine (Sync/Scalar/GpSimd) feeds its own HW
        # DMA ring; a ring round-robins packets across ALL in-flight
        # transfers, so concurrent chunks finish together at the END.
        # Every ring is therefore dep-CHAINED (in-flight=1) so chunks
        # complete in consumption order at full ring rate (~4KB rows).
        # w0 streams k-major as full-kp 512KB chunks: even kp on Sync,
        # odd kp on Scalar, giving ~2 chunks per 3.3us against phase
        # A's 1.7us/kp consumption.
        # xq: two 4KB-row halves chained on GpSimd (the first gates the
        # first matmul at ~12us, in parallel with w0-kp0). gb (32B,
        # latency-dominated) launches unchained so it never blocks a
        # chain. Measured ring facts: ~130 GB/s per ring single
        # transfer, ~180-200 with two in flight; rows <2KB crater.
        d_xqa = nc.gpsimd.dma_start(xq_sb[:, 0:8, :], xq[:, 0:8, :])
        d_gb = nc.gpsimd.dma_start(gb_sb[:], gb[:, :])
        d_xqb = nc.gpsimd.dma_start(xq_sb[:, 8:DS, :], xq[:, 8:DS, :])
        add_dep_helper(d_xqb.ins, d_xqa.ins, reason="xq ring chain")
        # w0: even kp chunks on Sync, odd on Scalar, lag-1 chained so
        # each ring keeps 2 transfers in flight (full ring rate) while
        # completions stay in consumption order.
        w_sb0 = wpool.tile([128, WJ, 1024], dt.float8e4, tag="w")
        d_w0 = {}
        for kp in range(KP):
            eng = nc.sync if kp % 2 == 0 else nc.scalar
            dj = eng.dma_start(
                w_sb0[:, 4 * kp:4 * kp + 4, :],
                wq[:, 4 * kp:4 * kp + 4, :])
            if kp >= 4:
                add_dep_helper(dj.ins, d_w0[kp - 4].ins,
                               reason="w0 ring lag-1 chain")
            d_w0[kp] = dj
        # gwt (64KB but 256B rows, ~5us) after w0 on Scalar; xf half 0
        # on GpSimd gated after w0; xf half 1 on Scalar after gwt. All
        # land >3us before the gating block consumes them.
        d_gwt = nc.scalar.dma_start(gwt_sb[:], gwt[:, :, :])
        add_dep_helper(d_gwt.ins, d_w0[7].ins, reason="scalar ring chain")
        d_xf = []
        dj = nc.gpsimd.dma_start(xf_sb[:, 0:8, :], xf[:, 0:8, :])
        add_dep_helper(dj.ins, d_xqb.ins, reason="gpsimd ring chain")
        add_dep_helper(dj.ins, d_w0[6].ins, reason="xf after w0 sync ring")
        add_dep_helper(dj.ins, d_w0[7].ins, reason="xf after w0 scalar ring")
        d_xf.append(dj)
        dj = nc.scalar.dma_start(xf_sb[:, 8:DS, :], xf[:, 8:DS, :])
        add_dep_helper(dj.ins, d_gwt.ins, reason="scalar ring chain")
        d_xf.append(dj)

        def rhs_ap(w_sb, kp, hc):
            j = 4 * kp + 2 * (hc // 2)
            q = hc % 2
            return w_sb[:, j:j + 2, q * 512:(q + 1) * 512]

        # Split PSUM->acc copies alternately across ACT and DVE so each
        # phase's copy chain drains twice as fast.
        def copy_out(i, dst, src):
            if i % 2 == 0:
                nc.scalar.copy(dst, src)
            else:
                nc.vector.tensor_copy(dst, src)

        # ---- Expert 0: m-split phases, k-major consumption ----
        # Phase A (mc0,1 x hc0-3) consumes w0 chunks as they arrive;
        # phase B (mc2,3) replays them from SBUF. PSUM -> acc UNSCALED.
        def e0_phase(mcs):
            pss = {
                mc: [psp.tile([128, NH], dt.float32, tag="ps",
                              name=f"ps0_{mc}_{hc}") for hc in range(HC)]
                for mc in mcs
            }
            for kp in range(KP):
                for mc in mcs:
                    lhsT = xq_sb[:, 2 * kp:2 * kp + 2, mc * 128:(mc + 1) * 128]
                    for hc in range(HC):
                        nc.tensor.matmul(
                            pss[mc][hc][:],
                            lhsT=lhsT,
                            rhs=rhs_ap(w_sb0, kp, hc),
                            start=(kp == 0),
                            stop=(kp == KP - 1),
                            perf_mode=mybir.MatmulPerfMode.DoubleRow,
                        )
            i = 0
            for mc in mcs:
                for hc in range(HC):
                    copy_out(i, acc_sb[:, mc * H + hc * NH:mc * H + (hc + 1) * NH],
                             pss[mc][hc][:])
                    i += 1

        e0_phase((0, 1))
        e0_phase((2, 3))

        # ---- Gating matmuls right after phase B (they absorb phase
        # B's copy-chain drain before e1's matmuls need those banks).
        ps_gt = psp.tile([E, MS], dt.float32, tag="ps", name="ps_gt")
        for s in range(DS):
            nc.tensor.matmul(
                ps_gt[:],
                lhsT=gwt_sb[:, s:s + 1, :],
                rhs=xf_sb[:, s:s + 1, :],
                start=(s == 0),
                stop=(s == DS - 1),
            )
        nc.vector.tensor_scalar_add(lg_sb[:], ps_gt[:], gb_sb[:])

        def emit_softmax():
            for mc in range(MC):
                pst = psp.tile([128, E], dt.float32, tag="ps", name=f"ps_t{mc}")
                nc.tensor.transpose(
                    pst[:], lg_sb[:, mc * 128:(mc + 1) * 128], id8_sb[:]
                )
                mx = small.tile([128, 1], dt.float32, tag="sm1")
                nc.vector.tensor_reduce(
                    mx[:], pst[:], mybir.AxisListType.X, mybir.AluOpType.max
                )
                nmx = small.tile([128, 1], dt.float32, tag="sm1")
                nc.vector.tensor_scalar_mul(nmx[:], mx[:], -1.0)
                ex = small.tile([128, E], dt.float32, tag="sm")
                ssum = small.tile([128, 1], dt.float32, tag="sm1")
                nc.scalar.activation(
                    ex[:], pst[:], mybir.ActivationFunctionType.Exp,
                    bias=nmx[:], scale=1.0, accum_out=ssum[:],
                )
                rcp = small.tile([128, 1], dt.float32, tag="sm1")
                nc.vector.reciprocal(rcp[:], ssum[:])
                nc.vector.tensor_scalar_mul(
                    gate_sb[:, mc * E:(mc + 1) * E], ex[:], rcp[:]
                )

        # ---- Experts 1..7: mc-major, DVE combine straight from PSUM ----
        # Output-launch queues: GpSimd only gets early tiles (its
        # end-of-kernel queue drain would otherwise serialize the
        # teardown behind a late transfer).
        out_q = {
            0: [nc.gpsimd, nc.gpsimd, nc.gpsimd, nc.gpsimd],
            1: [nc.scalar, nc.scalar, nc.scalar, nc.scalar],
            2: [nc.sync, nc.sync, nc.gpsimd, nc.scalar],
            3: [nc.scalar, nc.sync, nc.scalar, nc.sync],
        }
        sync_prev = d_w0[6]
        scalar_prev = d_xf[1]
        for e in range(1, E):
            w_sb = wpool.tile([128, WJ, 1024], dt.float8e4, tag="w")
            if e == 1:
                # e1's first half in two 1MB chunks (so early k-groups
                # gate on kp0-1, not a 2MB transfer-end semaphore).
                for c in range(2):
                    dw = nc.sync.dma_start(
                        w_sb[:, 8 * c:8 * c + 8, :],
                        wq[:, (KP + 2 * c) * 4:(KP + 2 * c + 2) * 4, :])
                    add_dep_helper(dw.ins, sync_prev.ins,
                                   reason="sync ring chain")
                    sync_prev = dw
            else:
                dw = nc.sync.dma_start(
                    w_sb[:, 0:16, :],
                    wq[:, e * WJ:e * WJ + 16, :])
                add_dep_helper(dw.ins, sync_prev.ins, reason="sync ring chain")
                sync_prev = dw
            dw = nc.scalar.dma_start(
                w_sb[:, 16:32, :],
                wq[:, e * WJ + 16:(e + 1) * WJ, :])
            add_dep_helper(dw.ins, scalar_prev.ins, reason="scalar ring chain")
            scalar_prev = dw
            for mc in range(MC):
                if e == 1 and mc == 1:
                    # Rotation padding: the softmax block inserted 5
                    # PSUM allocations (ps_gt + 4 transposes), breaking
                    # the 4-slot alternation between mc-groups. Four
                    # pad slots (with DVE memsets emitted AFTER mc0's
                    # combines, so the FIFO has no cycle) realign the
                    # ring: every matmul group again lands on banks
                    # freed a full window earlier.
                    for p in range(4):
                        pad = psp.tile([128, 1], dt.float32, tag="ps",
                                       name=f"ps_pad{p}")
                        nc.vector.memset(pad[:], 0.0)
                msl = slice(mc * 128, (mc + 1) * 128)
                pss = [
                    psp.tile([128, NH], dt.float32, tag="ps", name=f"ps_{e}_{mc}_{i}")
                    for i in range(HC)
                ]
                g_ap = gate_sb[:, mc * E + e:mc * E + e + 1]
                if e == E - 1 and mc == MC - 1:
                    # Final group hc-major: each hc tile's combine+DMA
                    # chases the matmul stream; only the last half-tile
                    # trails the last matmul.
                    for hc in range(HC):
                        for k in range(KP):
                            nc.tensor.matmul(
                                pss[hc][:],
                                lhsT=xq_sb[:, 2 * k:2 * k + 2, msl],
                                rhs=rhs_ap(w_sb, k, hc),
                                start=(k == 0),
                                stop=(k == KP - 1),
                                perf_mode=mybir.MatmulPerfMode.DoubleRow,
                            )
                        a_ap = acc_sb[:, mc * H + hc * NH:mc * H + (hc + 1) * NH]
                        if hc < HC - 1:
                            ot = outp.tile([128, NH], dt.bfloat16, tag="ot")
                            nc.vector.scalar_tensor_tensor(
                                ot[:], pss[hc][:], g_ap, a_ap,
                                op0=mybir.AluOpType.mult,
                                op1=mybir.AluOpType.add,
                            )
                            out_q[mc][hc].dma_start(
                                out[msl, hc * NH:(hc + 1) * NH], ot[:]
                            )
                        else:
                            ot = outp.tile([128, NH], dt.bfloat16, tag="ot")
                            for half, q in ((0, nc.scalar), (1, nc.sync)):
                                csl = slice(half * 256, (half + 1) * 256)
                                nc.vector.scalar_tensor_tensor(
                                    ot[:, csl], pss[hc][:, csl], g_ap,
                                    a_ap[:, csl],
                                    op0=mybir.AluOpType.mult,
                                    op1=mybir.AluOpType.add,
                                )
                                q.dma_start(
                                    out[msl, hc * NH + half * 256:
                                        hc * NH + (half + 1) * 256],
                                    ot[:, csl],
                                )
                    continue
                for k in range(KP):
                    lhsT = xq_sb[:, 2 * k:2 * k + 2, msl]
                    for hc in range(HC):
                        nc.tensor.matmul(
                            pss[hc][:],
                            lhsT=lhsT,
                            rhs=rhs_ap(w_sb, k, hc),
                            start=(k == 0),
                            stop=(k == KP - 1),
                            perf_mode=mybir.MatmulPerfMode.DoubleRow,
                        )
                    if e == 1 and mc == 0 and k == 0:
                        # Softmax transposes here: the PE is one k-step
                        # into e1, lg_sb is ready, phase B's copies are
                        # drained — no PE wait.
                        emit_softmax()
                if e == 1:
                    # Deferred e0 gate scale, on ACT (activation Copy
                    # with per-partition scale) so the DVE stays free
                    # for the combines.
                    g0_ap = gate_sb[:, mc * E:mc * E + 1]
                    for hc in range(HC):
                        a_ap = acc_sb[:, mc * H + hc * NH:mc * H + (hc + 1) * NH]
                        nc.scalar.activation(
                            a_ap, a_ap, mybir.ActivationFunctionType.Copy,
                            scale=g0_ap,
                        )
                for hc in range(HC):
                    a_ap = acc_sb[:, mc * H + hc * NH:mc * H + (hc + 1) * NH]
                    if e < E - 1:
                        nc.vector.scalar_tensor_tensor(
                            a_ap, pss[hc][:], g_ap, a_ap,
                            op0=mybir.AluOpType.mult, op1=mybir.AluOpType.add,
                        )
                    else:
                        ot = outp.tile([128, NH], dt.bfloat16, tag="ot")
                        nc.vector.scalar_tensor_tensor(
                            ot[:], pss[hc][:], g_ap, a_ap,
                            op0=mybir.AluOpType.mult, op1=mybir.AluOpType.add,
                        )
                        out_q[mc][hc].dma_start(
                            out[msl, hc * NH:(hc + 1) * NH], ot[:]
                        )

    nc.compile()
    return nc


def _get_nc() -> bass.Bass:
    global _NC
    if _NC is None:
        _NC = _build_program()
    return _NC


def _prep_in_maps(x, gate_w, gate_b, expert_w):
    f8fn = ml_dtypes.float8_e4m3fn
    f8trn = ml_dtypes.float8_e4m3  # same bits as e4m3fn for |v| <= 240

    x = np.asarray(x, dtype=np.float32)
    gate_w = np.asarray(gate_w, dtype=np.float32)
    gate_b = np.asarray(gate_b, dtype=np.float32)
    expert_w = np.asarray(expert_w, dtype=np.float32)

    # x^T: [D, M]; quantized and bf16 (gating) copies.
    xT = np.ascontiguousarray(x.T)                       # [D, M] f32
    xT_bf = xT.astype(ml_dtypes.bfloat16)                # [D, M] bf16 (gating)
    xqT = xT.astype(f8fn).view(f8trn)                    # [D, M] fp8
    # expert_w [E, H, D] -> w^T per expert [E, D, H], quantized, packed
    # into the flat [128, j, 1024] device layout with
    # j = ((e*KP + kp)*2 + hh)*2 + r and d = (2*kp + r)*128 + p.
    wqT = np.ascontiguousarray(
        expert_w.transpose(0, 2, 1)
    ).astype(f8fn).view(f8trn)                           # [E, D, H]
    wq_flat = np.ascontiguousarray(
        wqT.reshape(E, KP, 2, 128, 2, 1024)
           .transpose(3, 0, 1, 4, 2, 5)
           .reshape(128, E * WJ, 1024)
    )
    gwt_flat = np.ascontiguousarray(
        gate_w.T.astype(ml_dtypes.bfloat16)
              .reshape(DS, 128, E).transpose(1, 0, 2)
    )
    gbb = np.ascontiguousarray(gate_b.reshape(E, 1))

    in_maps = []
    for c in range(NCORES):
        csl = slice(c * MS, (c + 1) * MS)
        xq_c = np.ascontiguousarray(
            xqT[:, csl].reshape(DS, 128, MS).transpose(1, 0, 2))
        xf_c = np.ascontiguousarray(
            xT_bf[:, csl].reshape(DS, 128, MS).transpose(1, 0, 2))
        in_maps.append({
            "xq": xq_c,
            "xf": xf_c,
            "wq": wq_flat,
            "gwt": gwt_flat,
            "gb": gbb,
        })
    return in_maps


def kernel(x, gate_w, gate_b, expert_w, _trace=False, _trace_kwargs=None):
    nc = _get_nc()
    in_maps = _prep_in_maps(x, gate_w, gate_b, expert_w)
    kw = {}
    if _trace:
        kw["trace"] = True
        kw.update(_trace_kwargs or {})
    res = run_bass_kernel_spmd(nc, in_maps, core_ids=list(range(NCORES)), **kw)
    outp = np.concatenate(
        [np.asarray(res.results[c]["out"]).astype(np.float32)
         for c in range(NCORES)],
        axis=0,
    )
    if _trace:
        return outp, res
    return outp
